# revision 1
# baseline (speedup 1.0000x reference)
"""Trainium2 8-core kernel for the AGI transformer block.

Sharding: 2-way data parallel over batch x 4-way tensor parallel over heads.
Core c: batch b=c//4, feature band g=c%4 (256 features = 4 main heads of 64 /
1 causal head of 256 / 1 meta head of 256).

Per core (band slice G = [256g, 256g+256)):
  - main attention: 4 heads, q pre-scaled 1/8, sigmoid(gate+aw) modulation
    folded into q per-head; rowsums via ones-column in augmented V (M=65);
    head pairs share the PE array via base-partition 0/64 row packing.
  - causal MHA head: hd=256, q pre-scaled 1/16; 0.9 blend folded into out-proj
    weight; main's 0.1-scaled ctx placed into the out-proj PSUM via a host-built
    placement matrix (keeps the SPMD program core-independent).
  - blend combine: ReduceScatter(add) -> own band [256,2048] (for the final
    0.85-term) + AllGather -> full blended ctx [1024,2048] (for meta).
  - meta MHA head: hd=256; 0.15*meta_out_w.T@out_w.T folded into one matrix.
  - final: outP = mowT.T@metaA + owT.T@band_ctx  (partial; host sums 4 cores).

Storage is bf16 (halves SBUF + HBM traffic); accumulation is f32 in PSUM.
Emission interleaves ACT-bound main attention with PE-bound causal attention
and meta projections so the TensorE stream stays dense (HAM stays warm).
Chunked tiles keep Tile's dependency tracking precise so collectives overlap.
"""

import ml_dtypes
import numpy as np

import concourse.mybir as mybir
import concourse.tile as tile
from concourse import bacc
from concourse.bass_utils import run_bass_kernel_spmd

F32 = mybir.dt.float32
BF16 = mybir.dt.bfloat16
AF = mybir.ActivationFunctionType
BF = ml_dtypes.bfloat16

B, S, D = 2, 2048, 1024
NCORES = 8
G = 4  # tensor-parallel group size
BAND = 256  # features per core
IC, NIC = 512, 4  # i-chunk (query) tiling
NJT = 16  # j tiles of 128
NKT = 8  # contraction tiles of 128 over D
CAUSAL_ACTIVE = 0.9
MW = ((0.9 - 0.8) / 0.2) * 0.3  # 0.15


def build_program():
    nc = bacc.Bacc("TRN2", target_bir_lowering=False, debug=False,
                   num_devices=NCORES)

    def din(name, shape):
        return nc.dram_tensor(name, shape, BF16, kind="ExternalInput").ap()

    xT = din("xT", [D, S])
    wqT = din("wqT", [D, BAND])
    wkT = din("wkT", [D, BAND])
    wvT = din("wvT", [D, 260])  # 4x(64 head cols + zero col for ones)
    gwT = din("gwT", [D, 4])
    selT = din("selT", [4, 512])  # 4 one-hot row-selector blocks [4,128]
    awc = nc.dram_tensor("awc", [1, 4], F32, kind="ExternalInput").ap()
    cqT = din("cqT", [D, BAND])
    ckT = din("ckT", [D, BAND])
    cvT = din("cvT", [D, BAND])
    cowT = din("cowT", [BAND, D])
    pcT = din("pcT", [BAND, D])  # placement matrix (0.1 at own band)
    pselT = din("pselT", [D, BAND])  # one-hot band row-selector (chunk 3)
    mqT = din("mqT", [D, BAND])
    mkT = din("mkT", [D, BAND])
    mvT = din("mvT", [D, BAND])
    mowT = din("mowT", [BAND, D])
    owT = din("owT", [BAND, D])
    outP = nc.dram_tensor("outP", [D, S], F32, kind="ExternalOutput").ap()

    groups = [[0, 1, 2, 3], [4, 5, 6, 7]]

    with tile.TileContext(nc) as tc:
        with (
            tc.tile_pool(name="wts", bufs=1) as wts,
            tc.tile_pool(name="act", bufs=1) as actp,
            tc.tile_pool(name="small", bufs=1) as small,
            tc.tile_pool(name="work", bufs=3) as work,
            tc.tile_pool(name="stat", bufs=2) as statp,
            tc.tile_pool(name="psE", bufs=3, space="PSUM") as psE,
            tc.tile_pool(name="psA", bufs=4, space="PSUM") as psA,
            tc.tile_pool(name="psR", bufs=1, space="PSUM") as psR,
            tc.tile_pool(name="dram", bufs=1, space="DRAM") as dram,
        ):
            def load_w(name, ap, cols, tag):
                t = wts.tile([128, NKT, cols], BF16, name=name, tag=tag)
                for kt in range(NKT):
                    nc.sync.dma_start(t[:, kt, :],
                                      ap[kt * 128:(kt + 1) * 128, :])
                return t

            def load_w2(name, ap, tag):  # [256, 1024] -> [128, 2, 1024]
                t = wts.tile([128, 2, D], BF16, name=name, tag=tag)
                for kt in range(2):
                    nc.sync.dma_start(t[:, kt, :],
                                      ap[kt * 128:(kt + 1) * 128, :])
                return t

            wq_sb = load_w("wq_sb", wqT, BAND, "wq")

            # xT per-kt tiles; tags pair them with later-stage tiles so the
            # SBUF slots time-share (xT dies before those are written)
            xtags = ["ctxC0", "ctxC1", "ctxC2", "ctxC3",
                     "qT2", "kT2", "vA2", "bandC0"]
            xT_t = []
            for kt in range(NKT):
                t = actp.tile([128, S], BF16, name=f"xTt{kt}", tag=xtags[kt])
                nc.sync.dma_start(t[:, :], xT[kt * 128:(kt + 1) * 128, :])
                xT_t.append(t)

            wk_sb = load_w("wk_sb", wkT, BAND, "wk")
            wv_sb = load_w("wv_sb", wvT, 260, "wv")
            gw_sb = load_w("gw_sb", gwT, 4, "gw")
            cq_sb = load_w("cq_sb", cqT, BAND, "cq")
            ck_sb = load_w("ck_sb", ckT, BAND, "ck")
            cv_sb = load_w("cv_sb", cvT, BAND, "cv")
            cow_sb = load_w2("cow_sb", cowT, "cow")
            pc_sb = load_w2("pc_sb", pcT, "pc")
            mow_sb = load_w2("mow_sb", mowT, "mow")  # own slots: load early
            ow_sb = load_w2("ow_sb", owT, "ow")

            aw_sb = small.tile([4, 1], F32)
            nc.sync.dma_start(aw_sb[:, :], awc.rearrange("a b -> b a"))
            sel_sb = small.tile([4, 512], BF16)
            nc.sync.dma_start(sel_sb[:, :], selT[:, :])
            ones_sb = small.tile([128, 1], BF16)
            nc.vector.memset(ones_sb[:, :], 1.0)
            onesrow = small.tile([1, 128], BF16)
            nc.vector.memset(onesrow[:, :], 1.0)

            def pe_keepwarm(n=8):
                # dependency-free LDWEIGHTS burst: keeps the PE activity
                # monitor busy across division-chain waits (no PSUM needed)
                for _ in range(n):
                    nc.tensor.ldweights(kT_sb[:, 0, 0:128])

            # ---------- projections ----------
            qT_sb = actp.tile([128, 2, S], BF16, tag="qT")
            kT_sb = actp.tile([128, 2, S], BF16, tag="kT")

            def proj_chunk(dst, w_sb, src_t, ot, icc):
                ps = psA.tile([128, IC], F32, tag="acc")
                for kt in range(NKT):
                    nc.tensor.matmul(
                        ps[:, :],
                        w_sb[:, kt, ot * 128:(ot + 1) * 128],
                        src_t[kt][:, icc * IC:(icc + 1) * IC],
                        start=(kt == 0), stop=(kt == NKT - 1))
                nc.vector.tensor_copy(dst[:, ot, icc * IC:(icc + 1) * IC],
                                      ps[:, :])

            def proj_T(dst, w_sb, src_t):  # dst [128, 2, S]
                for ot in range(2):
                    for icc in range(NIC):
                        proj_chunk(dst, w_sb, src_t, ot, icc)

            # gate matmuls + sigmoid for ALL chunks now (keeps the sigmoid
            # table-set switch out of the attention blocks)
            mrow4 = small.tile([4, S], BF16)
            for icc in range(NIC):
                i0 = icc * IC
                g_ps = psR.tile([4, IC], F32, tag="rs")
                for kt in range(NKT):
                    nc.tensor.matmul(g_ps[:, :],
                                     gw_sb[:, kt, 0:4],
                                     xT_t[kt][:, i0:i0 + IC],
                                     start=(kt == 0), stop=(kt == NKT - 1))
                nc.scalar.activation(mrow4[:, i0:i0 + IC], g_ps[:, :],
                                     AF.Sigmoid, bias=aw_sb[:, 0:1], scale=1.0)

            qs_sb = actp.tile([128, 2, 2 * IC], BF16, tag="qs")  # 2-chunk ring

            def qmod(h, icc):
                # broadcast row h of mrow4 to 128 partitions via a K=4 matmul
                # against a host-provided one-hot selector, then scale q into
                # a separate tile (avoids write-after-read on qT)
                rh, oh = (h % 2) * 64, h // 2
                i0 = icc * IC
                pb = psR.tile([128, IC], F32, tag="rs")
                nc.tensor.matmul(pb[:, :],
                                 sel_sb[0:4, h * 128:(h + 1) * 128],
                                 mrow4[0:4, i0:i0 + IC])
                r0 = (icc % 2) * IC
                nc.vector.tensor_mul(qs_sb[rh:rh + 64, oh, r0:r0 + IC],
                                     qT_sb[rh:rh + 64, oh, i0:i0 + IC],
                                     pb[rh:rh + 64, :])

            def qproj_steps(icc):
                """q + cq projection/modulation filler steps for chunk icc"""
                steps = []
                for ot in range(2):
                    def sq(ot=ot, icc=icc):
                        proj_chunk(qT_sb, wq_sb, xT_t, ot, icc)
                        qmod(2 * ot, icc)
                        qmod(2 * ot + 1, icc)
                    steps.append(sq)
                for ot in range(2):
                    steps.append(lambda ot=ot, icc=icc: proj_chunk(
                        cqT_sb, cq_sb, xT_t, ot, icc))
                return steps

            # stage B: only chunk 0 of q/cq; full k/v/ck/cv
            cqT_sb = actp.tile([128, 2, S], BF16, tag="cqT")
            for st_ in qproj_steps(0):
                st_()
            proj_T(kT_sb, wk_sb, xT_t)

            # v natural layout [2048 j, 260]: ones cols at 64,129,194,259
            v_sb = actp.tile([128, NJT, 260], BF16, tag="vA")
            for st in range(NJT):
                ps = psA.tile([128, 260], F32, tag="acc")
                for kt in range(NKT):
                    nc.tensor.matmul(ps[:, :],
                                     xT_t[kt][:, st * 128:(st + 1) * 128],
                                     wv_sb[:, kt, :],
                                     start=(kt == 0), stop=(kt == NKT - 1))
                nc.vector.tensor_copy(v_sb[:, st, :], ps[:, :])
                nc.vector.memset(v_sb[:, st, 64:260:65], 1.0)

            ckT_sb = actp.tile([128, 2, S], BF16, tag="ckT")
            proj_T(ckT_sb, ck_sb, xT_t)

            cv_nat = actp.tile([128, NJT, BAND], BF16, tag="cvN")
            for st in range(NJT):
                ps = psA.tile([128, BAND], F32, tag="acc")
                for kt in range(NKT):
                    nc.tensor.matmul(ps[:, :],
                                     xT_t[kt][:, st * 128:(st + 1) * 128],
                                     cv_sb[:, kt, :],
                                     start=(kt == 0), stop=(kt == NKT - 1))
                nc.vector.tensor_copy(cv_nat[:, st, :], ps[:, :])

            # meta weights: load now (slots of wq/wk/wv just freed; sync queue
            # still shallow, so they land long before the meta stage)
            mq_sb = load_w("mq_sb", mqT, BAND, "wq")
            mk_sb = load_w("mk_sb", mkT, BAND, "wk")
            mv_sb = load_w("mv_sb", mvT, BAND, "wv")

            # ---------- chunked tiles ----------
            ctxm_sb = actp.tile([128, 2, S], BF16, tag="ctxm")  # 0.1*main ctx
            cA_sb = actp.tile([128, 2, S], BF16, tag="cA")
            ctxC = [actp.tile([128, NKT, IC], BF16, name=f"ctxC{i}",
                              tag=f"ctxC{i}") for i in range(NIC)]
            bandC = [actp.tile([128, 2, IC], BF16, name=f"bandC{i}",
                               tag=f"bandC{i}") for i in range(NIC)]
            mqT_sb = actp.tile([128, 2, S], BF16, tag="qT2")
            mkT_sb = actp.tile([128, 2, S], BF16, tag="kT2")
            mv_nat = actp.tile([128, NJT, BAND], BF16, tag="vA2")

            arB, rsO, agO, arO3 = [], [], [], []
            for icc in range(NIC):
                nh = 1 if icc < NIC - 1 else 2
                arB.append([dram.tile([D, IC // nh], BF16,
                                      name=f"arB{icc}_{hh}", tag=f"arB{icc}{hh}")
                            for hh in range(nh)])
                rsO.append([dram.tile([BAND, IC // nh], BF16,
                                      name=f"rsO{icc}_{hh}", tag=f"rsO{icc}{hh}")
                            for hh in range(nh)])
                if icc == NIC - 1:
                    arO3.extend([dram.tile([D, IC // nh], BF16,
                                           name=f"arO3_{hh}", tag=f"arO3{hh}")
                                 for hh in range(nh)])
                agO.append([dram.tile([D, IC // nh], BF16,
                                      name=f"agO{icc}_{hh}", tag=f"agO{icc}{hh}")
                            for hh in range(nh)])

            def div_batch(specs, i0):
                """batched softmax divisions: pipeline ln -> exp -> bcast ->
                muls across several heads so chain latencies overlap.
                spec: ("head", h, acc) or ("wide", dst_sb, a1, a2, rs)"""
                lnrs = []
                for sp in specs:
                    lnr = statp.tile([1, IC], F32, tag="lnr", bufs=3)
                    src = sp[2][64:65, :] if sp[0] == "head" else sp[4][:, :]
                    nc.scalar.activation(lnr[:, :], src, AF.Ln)
                    lnrs.append(lnr)
                rcps = []
                for sp, lnr in zip(specs, lnrs):
                    rcp = statp.tile([1, IC], BF16, tag="rcp", bufs=3)
                    nc.scalar.activation(rcp[:, :], lnr[:, :], AF.Exp,
                                         scale=-1.0)
                    rcps.append(rcp)
                pbs = []
                for sp, rcp in zip(specs, rcps):
                    n = 64 if sp[0] == "head" else 128
                    # wide-head broadcasts use the psR bank (freed by the ln
                    # that read the rowsum); head broadcasts rotate via eps
                    if sp[0] == "head":
                        pb_ps = psE.tile([128, IC], F32, tag="eps")
                    else:
                        pb_ps = psR.tile([128, IC], F32, tag="rs")
                    nc.tensor.matmul(pb_ps[:, :], onesrow[0:1, :], rcp[:, :])
                    pb = work.tile([n, IC], BF16,
                                   tag="pbm" if n == 64 else "pb2", bufs=3)
                    nc.scalar.copy(pb[:, :], pb_ps[0:n, :])
                    pbs.append(pb)
                for sp, pb in zip(specs, pbs):
                    if sp[0] == "head":
                        h, acc = sp[1], sp[2]
                        rh, oh = (h % 2) * 64, h // 2
                        nc.vector.tensor_mul(
                            ctxm_sb[rh:rh + 64, oh, i0:i0 + IC],
                            acc[0:64, :], pb[:, :])
                    else:
                        dst_sb, a1, a2 = sp[1], sp[2], sp[3]
                        nc.vector.tensor_mul(dst_sb[:, 0, i0:i0 + IC],
                                             a1[:, :], pb[:, :])
                        nc.vector.tensor_mul(dst_sb[:, 1, i0:i0 + IC],
                                             a2[:, :], pb[:, :])

            def main_pair_step(p, jt, i0, accs):
                """one j-tile for main head pair p (heads 2p, 2p+1): the two
                E matmuls row-pack (base partitions 0/64) and run concurrently"""
                oh = p
                esbs = []
                for hh in range(2):
                    rh = hh * 64
                    eps = psE.tile([128, IC], F32, tag="eps")
                    r0 = (i0 // IC % 2) * IC
                    nc.tensor.matmul(
                        eps[:, :],
                        kT_sb[rh:rh + 64, oh, jt * 128:(jt + 1) * 128],
                        qs_sb[rh:rh + 64, oh, r0:r0 + IC])
                    esb = work.tile([128, IC], BF16, tag="esb", bufs=4)
                    nc.scalar.activation(esb[:, :], eps[:, :], AF.Exp)
                    esbs.append(esb)
                for hh in range(2):
                    h = 2 * p + hh
                    nc.tensor.matmul(
                        accs[hh][:, :],
                        v_sb[:, jt, h * 65:h * 65 + 65],
                        esbs[hh][:, :],
                        start=(jt == 0), stop=(jt == NJT - 1))

            def wide_attn_step(kTt, qTt, vnat, jt, i0, a1, a2, rs):
                """one j-tile of a hd-256 attention (causal or meta)"""
                eps = psE.tile([128, IC], F32, tag="eps")
                for dkt in range(2):
                    nc.tensor.matmul(
                        eps[:, :],
                        kTt[:, dkt, jt * 128:(jt + 1) * 128],
                        qTt[:, dkt, i0:i0 + IC],
                        start=(dkt == 0), stop=(dkt == 1))
                esb = work.tile([128, IC], BF16, tag="esb", bufs=4)
                nc.scalar.activation(esb[:, :], eps[:, :], AF.Exp)
                st_, sp_ = (jt == 0), (jt == NJT - 1)
                nc.tensor.matmul(a1[:, :], vnat[:, jt, 0:128], esb[:, :],
                                 start=st_, stop=sp_)
                nc.tensor.matmul(a2[:, :], vnat[:, jt, 128:256], esb[:, :],
                                 start=st_, stop=sp_)
                nc.tensor.matmul(rs[:, :], ones_sb[:, 0:1], esb[:, :],
                                 start=st_, stop=sp_)

            def metaproj_steps(icc):
                """closures, each emitting one PSUM group of chunk icc's meta
                projections (interleaved into phase 2 of chunk icc+1)"""
                i0 = icc * IC
                steps = []

                def projstep(w_sb, dst, ot, i0=i0, icc=icc):
                    ps = psA.tile([128, IC], F32, tag="acc")
                    for kt in range(NKT):
                        nc.tensor.matmul(
                            ps[:, :], w_sb[:, kt, ot * 128:(ot + 1) * 128],
                            ctxC[icc][:, kt, :],
                            start=(kt == 0), stop=(kt == NKT - 1))
                    nc.vector.tensor_copy(dst[:, ot, i0:i0 + IC], ps[:, :])

                def vstep(st4, icc=icc):
                    st = icc * 4 + st4
                    ps = psA.tile([128, BAND], F32, tag="acc")
                    for kt in range(NKT):
                        nc.tensor.matmul(
                            ps[:, :],
                            ctxC[icc][:, kt, st4 * 128:(st4 + 1) * 128],
                            mv_sb[:, kt, :],
                            start=(kt == 0), stop=(kt == NKT - 1))
                    nc.vector.tensor_copy(mv_nat[:, st, :], ps[:, :])

                for ot in range(2):
                    steps.append(lambda ot=ot: projstep(mq_sb, mqT_sb, ot))
                for ot in range(2):
                    steps.append(lambda ot=ot: projstep(mk_sb, mkT_sb, ot))
                for st4 in range(4):
                    steps.append(lambda st4=st4: vstep(st4))
                return steps

            # ---------- per-chunk pipeline ----------
            pending_rdback = []
            for icc in range(NIC):
                i0 = icc * IC
                while pending_rdback:
                    pending_rdback.pop(0)()
                # phase 1: main heads (0,1) interleaved with causal attention
                accA = [psA.tile([65, IC], F32, tag="acc", name=f"accA{icc}{i}")
                        for i in range(2)]
                ca1 = psA.tile([128, IC], F32, tag="acc")
                ca2 = psA.tile([128, IC], F32, tag="acc")
                crs = psR.tile([1, IC], F32, tag="rs")
                for jt in range(NJT):
                    main_pair_step(0, jt, i0, accA)
                    wide_attn_step(ckT_sb, cqT_sb, cv_nat, jt, i0, ca1, ca2, crs)
                div_batch([("head", 0, accA[0]), ("head", 1, accA[1]),
                           ("wide", cA_sb, ca1, ca2, crs)], i0)
                pe_keepwarm()

                # phase 2: main heads (2,3) interleaved with filler PE work:
                # next chunk's q/cq projections + older chunk's meta projs
                accB = [psA.tile([65, IC], F32, tag="acc", name=f"accB{icc}{i}")
                        for i in range(2)]
                fillers = []
                if icc + 1 < NIC:
                    fillers += qproj_steps(icc + 1)
                if icc >= 2:
                    fillers += metaproj_steps(icc - 2)
                # weave ~2/3 of the fillers into rounds 0..11 so the pair-B
                # accumulators stop promptly; the rest interleave with the
                # causal out-proj below (keeps PE dense across the boundary)
                nfront = len(fillers)
                done = 0
                for jt in range(NJT):
                    main_pair_step(1, jt, i0, accB)
                    want = (jt + 1) * nfront // NJT
                    while done < want:
                        fillers[done]()
                        done += 1
                div_batch([("head", 2, accB[0]), ("head", 3, accB[1])], i0)
                pe_keepwarm()

                # causal out-proj + main placement -> arB chunk
                for ot in range(8):
                    ps = psA.tile([128, IC], F32, tag="acc")
                    for ft in range(2):
                        nc.tensor.matmul(
                            ps[:, :],
                            cow_sb[:, ft, ot * 128:(ot + 1) * 128],
                            cA_sb[:, ft, i0:i0 + IC],
                            start=(ft == 0), stop=False)
                    for rt in range(2):
                        nc.tensor.matmul(
                            ps[:, :],
                            pc_sb[:, rt, ot * 128:(ot + 1) * 128],
                            ctxm_sb[:, rt, i0:i0 + IC],
                            start=False, stop=(rt == 1))
                    ob = work.tile([128, IC], BF16, tag="obA", bufs=2)
                    nc.vector.tensor_copy(ob[:, :], ps[:, :])
                    nh = len(arB[icc])
                    hw_ = IC // nh
                    for hh in range(nh):
                        nc.sync.dma_start(
                            arB[icc][hh][ot * 128:(ot + 1) * 128, :],
                            ob[:, hh * hw_:(hh + 1) * hw_])
                    if done < len(fillers) and ot % 2 == 1:
                        fillers[done]()
                        done += 1

                # blend combine: RS (own band) + AG (full ctx); the last
                # chunk is split in half so its latency tail is shorter.
                # Read-back DMAs are DEFERRED one block so the sync queue
                # never parks on an unfinished collective (head-of-line).
                nh = len(arB[icc])
                hw_ = IC // nh
                for hh in range(nh):
                    c0 = hh * hw_
                    if icc < NIC - 1:
                        nc.gpsimd.collective_compute(
                            "ReduceScatter", mybir.AluOpType.add,
                            replica_groups=groups,
                            ins=[arB[icc][hh][:, :].opt()],
                            outs=[rsO[icc][hh][:, :].opt()])
                        nc.gpsimd.collective_compute(
                            "AllGather", mybir.AluOpType.bypass,
                            replica_groups=groups,
                            ins=[rsO[icc][hh][:, :].opt()],
                            outs=[agO[icc][hh][:, :].opt()])
                    else:
                        # last chunk: single AllReduce per half (shorter
                        # serial chain on the collective engine); own band
                        # recovered by a select matmul in the tail
                        nc.gpsimd.collective_compute(
                            "AllReduce", mybir.AluOpType.add,
                            replica_groups=groups,
                            ins=[arB[icc][hh][:, :].opt()],
                            outs=[arO3[hh][:, :].opt()])

                    def rdback(icc=icc, hh=hh, c0=c0, hw_=hw_):
                        src = agO[icc][hh] if icc < NIC - 1 else arO3[hh]
                        for kt in range(NKT):
                            nc.sync.dma_start(
                                ctxC[icc][:, kt, c0:c0 + hw_],
                                src[kt * 128:(kt + 1) * 128, :])
                        if icc < NIC - 1:
                            for kt in range(2):
                                nc.sync.dma_start(
                                    bandC[icc][:, kt, c0:c0 + hw_],
                                    rsO[icc][hh][kt * 128:(kt + 1) * 128, :])
                    pending_rdback.append(rdback)

            # meta projections for the last two chunks (chunk 2 overlaps
            # the RS3/AG3 tail; chunk 3 is emitted inside the first meta
            # attention chunk below, after its early j-tiles)
            while pending_rdback:
                pending_rdback.pop(0)()
            for st in metaproj_steps(NIC - 2):
                st()

            # ---------- meta attention + final out-proj ----------
            def final_steps(icc):
                i0 = icc * IC
                steps = []

                def fstep(ot, icc=icc, i0=i0):
                    ps = psA.tile([128, IC], F32, tag="acc")
                    for ft in range(2):
                        nc.tensor.matmul(
                            ps[:, :],
                            mow_sb[:, ft, ot * 128:(ot + 1) * 128],
                            mA_sb[:, ft, i0:i0 + IC],
                            start=(ft == 0), stop=False)
                    for ft in range(2):
                        nc.tensor.matmul(
                            ps[:, :],
                            ow_sb[:, ft, ot * 128:(ot + 1) * 128],
                            bandC[icc][:, ft, :],
                            start=False, stop=(ft == 1))
                    ob = work.tile([128, IC], F32, tag="obF", bufs=2)
                    nc.vector.tensor_copy(ob[:, :], ps[:, :])
                    nc.sync.dma_start(
                        outP[ot * 128:(ot + 1) * 128, i0:i0 + IC], ob[:, :])
                for ot in range(8):
                    steps.append(lambda ot=ot: fstep(ot))
                return steps

            mA_sb = actp.tile([128, 2, S], BF16, tag="cqT")  # reuse slot
            for icc in range(NIC):
                i0 = icc * IC
                fsteps = final_steps(icc - 1) if icc > 0 else []
                a1 = psA.tile([128, IC], F32, tag="acc")
                a2 = psA.tile([128, IC], F32, tag="acc")
                rs = psR.tile([1, IC], F32, tag="rs")
                # j-tiles 0..11 only touch chunks 0-2 of mk/mv, so the first
                # i-chunk's early j-tiles run while chunk 3's AG completes;
                # chunk 3's meta projections emit before the last 4 j-tiles
                for jt in range(12):
                    wide_attn_step(mkT_sb, mqT_sb, mv_nat, jt, i0, a1, a2, rs)
                    if fsteps and jt % 2 == 1 and jt // 2 < len(fsteps):
                        fsteps[jt // 2]()
                if icc == 0:
                    psel_sb = load_w("psel_sb", pselT, BAND, "cq")
                    for st in metaproj_steps(NIC - 1):
                        st()
                    for rt in range(2):
                        ps = psA.tile([128, IC], F32, tag="acc")
                        for kt in range(NKT):
                            nc.tensor.matmul(
                                ps[:, :],
                                psel_sb[:, kt, rt * 128:(rt + 1) * 128],
                                ctxC[NIC - 1][:, kt, :],
                                start=(kt == 0), stop=(kt == NKT - 1))
                        nc.vector.tensor_copy(bandC[NIC - 1][:, rt, :],
                                              ps[:, :])
                for jt in range(12, NJT):
                    wide_attn_step(mkT_sb, mqT_sb, mv_nat, jt, i0, a1, a2, rs)
                    if fsteps and jt % 2 == 1 and jt // 2 < len(fsteps):
                        fsteps[jt // 2]()
                div_batch([("wide", mA_sb, a1, a2, rs)], i0)
                pe_keepwarm()

            for st in final_steps(NIC - 1):
                st()

    nc.compile()
    return nc


_NC = None


def _get_nc():
    global _NC
    if _NC is None:
        _NC = build_program()
    return _NC


def kernel(hidden_states, consciousness_vector, wq, bq, wk, bk, wv, bv,
           gate_w, gate_b, aw_w, aw_b,
           causal_in_w, causal_in_b, causal_out_w, causal_out_b,
           meta_in_w, meta_in_b, meta_out_w, meta_out_b,
           out_w, out_b):
    f = np.float32
    hs = np.asarray(hidden_states, f)
    aw = np.asarray(consciousness_vector, f) @ np.asarray(aw_w, f).T \
        + np.asarray(aw_b, f)
    wfused = np.asarray(meta_out_w, f).T @ np.asarray(out_w, f).T  # [D, D]
    xTs = [np.ascontiguousarray(hs[b].T).astype(BF) for b in range(B)]

    def bfT(a):  # transpose + bf16
        return np.ascontiguousarray(np.asarray(a, f).T).astype(BF)

    in_maps = []
    for c in range(NCORES):
        b, g = c // G, c % G
        sl = slice(g * BAND, (g + 1) * BAND)
        wv_aug = np.zeros((D, 260), f)
        for h in range(4):
            wv_aug[:, h * 65:h * 65 + 64] = \
                np.asarray(wv, f)[g * BAND + h * 64: g * BAND + (h + 1) * 64].T
        sel4 = np.zeros((4, 512), f)
        for h in range(4):
            sel4[h, h * 128:(h + 1) * 128] = 1.0
        sel4 = sel4.astype(BF)
        pc = np.zeros((BAND, D), f)
        pc[np.arange(BAND), g * BAND + np.arange(BAND)] = 0.1
        psel = np.zeros((D, BAND), f)
        psel[g * BAND + np.arange(BAND), np.arange(BAND)] = 1.0
        in_maps.append({
            "xT": xTs[b],
            "wqT": bfT(np.asarray(wq, f)[sl] / 8.0),
            "wkT": bfT(np.asarray(wk, f)[sl]),
            "wvT": wv_aug.astype(BF),
            "gwT": bfT(np.asarray(gate_w, f)[4 * g:4 * g + 4]),
            "selT": sel4,
            "awc": np.ascontiguousarray(aw[4 * g:4 * g + 4].reshape(1, 4)),
            "cqT": bfT(np.asarray(causal_in_w, f)[0:D][sl] / 16.0),
            "ckT": bfT(np.asarray(causal_in_w, f)[D:2 * D][sl]),
            "cvT": bfT(np.asarray(causal_in_w, f)[2 * D:][sl]),
            "cowT": np.ascontiguousarray(
                CAUSAL_ACTIVE * np.asarray(causal_out_w, f).T[sl]).astype(BF),
            "pcT": pc.astype(BF),
            "pselT": psel.astype(BF),
            "mqT": bfT(np.asarray(meta_in_w, f)[0:D][sl] / 16.0),
            "mkT": bfT(np.asarray(meta_in_w, f)[D:2 * D][sl]),
            "mvT": bfT(np.asarray(meta_in_w, f)[2 * D:][sl]),
            "mowT": np.ascontiguousarray(MW * wfused[sl]).astype(BF),
            "owT": np.ascontiguousarray(
                (1.0 - MW) * np.asarray(out_w, f).T[sl]).astype(BF),
        })

    nc = _get_nc()
    res = run_bass_kernel_spmd(nc, in_maps, core_ids=list(range(NCORES)))

    bias_row = (np.asarray(out_b, f)
                + MW * (np.asarray(meta_out_b, f) @ np.asarray(out_w, f).T))
    out = np.empty((B, S, D), f)
    for b in range(B):
        acc = np.zeros((D, S), f)
        for g in range(G):
            acc += res.results[b * G + g]["outP"]
        out[b] = acc.T + bias_row[None, :]
    return out



# revision 7
# speedup vs baseline: 1.2247x; 1.2247x over previous
"""Trainium2 8-core kernel for the AGI transformer block.

Sharding: 2-way data parallel over batch x 4-way tensor parallel over heads.
Core c: batch b=c//4, feature band g=c%4 (256 features = 4 main heads of 64 /
1 causal head of 256 / 1 meta head of 256).

Precision split by blend weight: the causal path (0.9) and the 0.85 final
out-proj stay bf16; the main path (0.1) and meta path (0.15) run fp8e4m3
with DoubleRow matmuls (two 128-deep contraction subtiles per instruction,
2 MACs/cycle), halving their PE stream time. fp8 operands are pre-scaled
(weights x16, ctx x8, psum casts x2^-k) to sit in e4m3's normal range; the
net scale is folded into the ACT Exp `scale` or the division multiply.

Per core (band slice G = [256g, 256g+256)):
  - main attention: 4 heads, sigmoid(gate+aw) modulation folded into q;
    rowsums via ones-column in the fp8 V (M=65); AV runs DoubleRow over
    j-tile pairs; softmax scale 1/8 folded into the Exp activation.
  - causal MHA head: hd=256 bf16, q pre-scaled 1/16; 0.9 blend folded into
    out-proj weight; main's ctx enters the same PSUM via a DoubleRow
    placement matmul (one-hot x 1/16, ctxm carries 1.6/rs).
  - blend combine: ReduceScatter(add) -> own band (0.85 term) + AllGather
    -> full ctx (meta). Softmax division uses DVE reciprocal_approx_fast
    (no ACT table switches) + a PE ones-matmul partition broadcast.
  - meta MHA head: hd=256 fp8 DoubleRow; 0.15*meta_out_w.T@out_w.T folded
    into one bf16 matrix.
  - final: outP = mowT.T@metaA + owT.T@band_ctx (partial; host sums 4).

Emission interleaves ACT-bound main attention with PE-bound causal attention
and meta projections so the TensorE stream stays dense.
"""

import os

import ml_dtypes
import numpy as np

DEBUG = os.environ.get("KDBG") == "1"

import concourse.mybir as mybir
import concourse.tile as tile
from concourse import bacc
from concourse.bass_utils import run_bass_kernel_spmd

F32 = mybir.dt.float32
BF16 = mybir.dt.bfloat16
F8 = mybir.dt.float8e4
AF = mybir.ActivationFunctionType
MUL = mybir.AluOpType.mult
DR = mybir.MatmulPerfMode.DoubleRow
BF = ml_dtypes.bfloat16
F8NP = ml_dtypes.float8_e4m3

B, S, D = 2, 2048, 1024
NCORES = 8
G = 4  # tensor-parallel group size
BAND = 256  # features per core
IC, NIC = 512, 4  # i-chunk (query) tiling
NJT = 16  # j tiles of 128
NPR = 8  # j-tile pairs per chunk
NKT = 8  # contraction tiles of 128 over D
CAUSAL_ACTIVE = 0.9
MW = ((0.9 - 0.8) / 0.2) * 0.3  # 0.15


def build_program():
    nc = bacc.Bacc("TRN2", target_bir_lowering=False, debug=False,
                   num_devices=NCORES)

    def din(name, shape, dt=BF16):
        return nc.dram_tensor(name, shape, dt, kind="ExternalInput").ap()

    xT = din("xT", [D, S])
    xf8T = din("xf8T", [D, S], F8)
    wqT = din("wqT", [D, BAND], F8)
    wkT = din("wkT", [D, BAND], F8)
    wvT = din("wvT", [D, 320], F8)  # 4x(64 head cols + ones slot + pad to 80)
    gwT = din("gwT", [D, 16], F8)  # 4 gate rows + zero pad
    selT = din("selT", [4, 512])  # 4 one-hot row-selector blocks [4,128]
    awc = nc.dram_tensor("awc", [1, 4], F32, kind="ExternalInput").ap()
    cqT = din("cqT", [D, BAND])
    ckT = din("ckT", [D, BAND])
    cvT = din("cvT", [D, BAND])
    cowT = din("cowT", [BAND, D])
    pcT = din("pcT", [BAND, D], F8)  # placement matrix (1/16 at own band)
    pselT = din("pselT", [D, BAND])  # one-hot band row-selector (chunk 3)
    mqT = din("mqT", [D, BAND], F8)
    mkT = din("mkT", [D, BAND], F8)
    mvT = din("mvT", [D, BAND], F8)
    mowT = din("mowT", [BAND, D])
    owT = din("owT", [BAND, D])
    outP = nc.dram_tensor("outP", [D, S], F32, kind="ExternalOutput").ap()
    dbg = {}
    if DEBUG:
        for nm, shape, dt in [
            ("d_mrow4", [4, S], BF16), ("d_kf8", [128, 2, S], F8),
            ("d_vsb", [128, NJT, 320], F8), ("d_ctxm", [128, 2, S], F8),
            ("d_cA", [128, 2, S], BF16), ("d_ctxC0", [128, NKT, IC], BF16),
            ("d_ctxF0", [128, NKT, IC], F8), ("d_mq", [128, 2, S], F8),
            ("d_mk", [128, 2, S], F8), ("d_mv", [128, NJT, BAND], F8),
            ("d_mA", [128, 2, S], BF16), ("d_bandC0", [128, 2, IC], BF16),
            ("d_qs", [128, 2, 2 * IC], F8),
        ]:
            dbg[nm] = nc.dram_tensor(nm, shape, dt,
                                     kind="ExternalOutput").ap()

    groups = [[0, 1, 2, 3], [4, 5, 6, 7]]

    with tile.TileContext(nc) as tc:
        with (
            tc.tile_pool(name="wts", bufs=1) as wts,
            tc.tile_pool(name="act", bufs=1) as actp,
            tc.tile_pool(name="small", bufs=1) as small,
            tc.tile_pool(name="work", bufs=3) as work,
            tc.tile_pool(name="stat", bufs=2) as statp,
            tc.tile_pool(name="psE", bufs=3, space="PSUM") as psE,
            tc.tile_pool(name="psA", bufs=4, space="PSUM") as psA,
            tc.tile_pool(name="psR", bufs=1, space="PSUM") as psR,
            tc.tile_pool(name="dram", bufs=1, space="DRAM") as dram,
        ):
            def load_w(name, ap, cols, tag, dt=BF16):
                t = wts.tile([128, NKT, cols], dt, name=name, tag=tag)
                for kt in range(NKT):
                    nc.sync.dma_start(t[:, kt, :],
                                      ap[kt * 128:(kt + 1) * 128, :])
                return t

            def load_w2(name, ap, tag, dt=BF16):  # [256, 1024] -> [128,2,1024]
                t = wts.tile([128, 2, D], dt, name=name, tag=tag)
                for kt in range(2):
                    nc.sync.dma_start(t[:, kt, :],
                                      ap[kt * 128:(kt + 1) * 128, :])
                return t

            wqf = load_w("wqf", wqT, BAND, "wq", F8)

            # fp8 x in DoubleRow layout, split in two tiles (kt 0-3 / 4-7)
            # whose slots are later reused by the meta ctx fp8 chunks 2/3
            xf8 = [actp.tile([128, 4, S], F8, name=f"xf8{i}", tag=f"xf8{i}")
                   for i in range(2)]
            for kt in range(NKT):
                nc.sync.dma_start(xf8[kt // 4][:, kt % 4, :],
                                  xf8T[kt * 128:(kt + 1) * 128, :])

            def xf8_sl(kp, c0, cw):  # kt-pair kp as [128, 2, cw] slice
                t, r = xf8[kp // 2], (kp % 2) * 2
                return t[:, r:r + 2, c0:c0 + cw]

            # bf16 x per-kt tiles (causal path); tags pair them with
            # later-stage tiles so the SBUF slots time-share
            xtags = ["ctxC0", "ctxC1", "ctxC2", "ctxC3",
                     "qT2", "kT2", "vA2", "bandC0"]
            xT_t = []
            for kt in range(NKT):
                t = actp.tile([128, S], BF16, name=f"xTt{kt}", tag=xtags[kt])
                nc.sync.dma_start(t[:, :], xT[kt * 128:(kt + 1) * 128, :])
                xT_t.append(t)

            wkf = load_w("wkf", wkT, BAND, "wk", F8)
            wvf = load_w("wvf", wvT, 320, "wv", F8)
            gwf = load_w("gwf", gwT, 16, "gw", F8)
            cq_sb = load_w("cq_sb", cqT, BAND, "cq")
            ck_sb = load_w("ck_sb", ckT, BAND, "ck")
            cv_sb = load_w("cv_sb", cvT, BAND, "cv")
            cow_sb = load_w2("cow_sb", cowT, "cow")
            pcf = load_w2("pcf", pcT, "pc", F8)
            mow_sb = load_w2("mow_sb", mowT, "mow")
            ow_sb = load_w2("ow_sb", owT, "ow")

            aw_sb = small.tile([4, 1], F32)
            nc.sync.dma_start(aw_sb[:, :], awc.rearrange("a b -> b a"))
            sel_sb = small.tile([4, 512], BF16)
            nc.sync.dma_start(sel_sb[:, :], selT[:, :])
            ones_sb = small.tile([128, 1], BF16)
            nc.vector.memset(ones_sb[:, :], 1.0)
            onesrow = small.tile([1, 128], BF16)
            nc.vector.memset(onesrow[:, :], 1.0)
            ones_pair = small.tile([128, 2, 16], F8)
            nc.vector.memset(ones_pair[:, :, :], 1.0)

            def pe_keepwarm(n=8):
                # dependency-free LDWEIGHTS burst across division waits
                for _ in range(n):
                    nc.tensor.ldweights(kf8_sb[:, 0, 0:128])

            # ---------- projections ----------
            qT_sb = actp.tile([128, 2, S], BF16, tag="qT")
            kf8_sb = actp.tile([128, 2, S], F8, tag="kT")

            def proj_chunk_dr(w_f8, ot, icc):
                ps = psA.tile([128, IC], F32, tag="acc")
                for kp in range(4):
                    nc.tensor.matmul(
                        ps[:, :],
                        w_f8[:, 2 * kp:2 * kp + 2, ot * 128:(ot + 1) * 128],
                        xf8_sl(kp, icc * IC, IC),
                        start=(kp == 0), stop=(kp == 3), perf_mode=DR)
                return ps

            def proj_chunk(dst, w_sb, src_t, ot, icc):  # bf16 (causal)
                ps = psA.tile([128, IC], F32, tag="acc")
                for kt in range(NKT):
                    nc.tensor.matmul(
                        ps[:, :],
                        w_sb[:, kt, ot * 128:(ot + 1) * 128],
                        src_t[kt][:, icc * IC:(icc + 1) * IC],
                        start=(kt == 0), stop=(kt == NKT - 1))
                nc.vector.tensor_copy(dst[:, ot, icc * IC:(icc + 1) * IC],
                                      ps[:, :])

            def proj_T(dst, w_sb, src_t):
                for ot in range(2):
                    for icc in range(4):
                        proj_chunk(dst, w_sb, src_t, ot, icc)

            # gate matmuls + sigmoid for ALL chunks now (keeps the sigmoid
            # table switch out of the attention blocks)
            mrow4 = small.tile([4, S], BF16)
            for icc in range(4):
                i0 = icc * IC
                g_ps = psR.tile([16, IC], F32, tag="rs")
                for kp in range(4):
                    nc.tensor.matmul(g_ps[:, :],
                                     gwf[:, 2 * kp:2 * kp + 2, 0:16],
                                     xf8_sl(kp, i0, IC),
                                     start=(kp == 0), stop=(kp == 3),
                                     perf_mode=DR)
                nc.scalar.activation(mrow4[:, i0:i0 + IC], g_ps[0:4, :],
                                     AF.Sigmoid, bias=aw_sb[:, 0:1],
                                     scale=1.0 / 16)

            qs_sb = actp.tile([128, 2, 2 * IC], F8, tag="qs")  # 2-chunk ring

            def qmod(h, icc):
                # broadcast row h of mrow4 to 128 partitions via a K=4 matmul
                # against a one-hot selector, then fold sigma into fp8 q
                rh, oh = (h % 2) * 64, h // 2
                i0 = icc * IC
                pb = psR.tile([128, IC], F32, tag="rs")
                nc.tensor.matmul(pb[:, :],
                                 sel_sb[0:4, h * 128:(h + 1) * 128],
                                 mrow4[0:4, i0:i0 + IC])
                r0 = (icc % 2) * IC
                nc.vector.tensor_mul(qs_sb[rh:rh + 64, oh, r0:r0 + IC],
                                     qT_sb[rh:rh + 64, oh, i0:i0 + IC],
                                     pb[rh:rh + 64, :])

            def qproj_steps(icc):
                """q + cq projection/modulation filler steps for chunk icc"""
                steps = []
                for ot in range(2):
                    def sq(ot=ot, icc=icc):
                        ps = proj_chunk_dr(wqf, ot, icc)
                        nc.vector.tensor_scalar_mul(
                            qT_sb[:, ot, icc * IC:(icc + 1) * IC], ps[:, :],
                            1.0 / 16)
                        qmod(2 * ot, icc)
                        qmod(2 * ot + 1, icc)
                    steps.append(sq)
                for ot in range(2):
                    steps.append(lambda ot=ot, icc=icc: proj_chunk(
                        cqT_sb, cq_sb, xT_t, ot, icc))
                return steps

            # stage B: only chunk 0 of q/cq; full k/v/ck/cv
            cqT_sb = actp.tile([128, 2, S], BF16, tag="cqT")
            for st_ in qproj_steps(0):
                st_()
            for ot in range(2):
                for icc in range(4):
                    ps = proj_chunk_dr(wkf, ot, icc)
                    nc.vector.tensor_scalar_mul(
                        kf8_sb[:, ot, icc * IC:(icc + 1) * IC], ps[:, :],
                        1.0 / 16)

            # v natural layout [2048 j, 320]: head h at cols 80h..80h+63,
            # ones at 80h+64 (written post-copy)
            v_sb = actp.tile([128, NJT, 320], F8, tag="vA")
            for st in range(NJT):
                ps = psA.tile([128, 320], F32, tag="acc")
                for kp in range(4):
                    nc.tensor.matmul(ps[:, :],
                                     xf8_sl(kp, st * 128, 128),
                                     wvf[:, 2 * kp:2 * kp + 2, :],
                                     start=(kp == 0), stop=(kp == 3),
                                     perf_mode=DR)
                nc.vector.tensor_scalar_mul(v_sb[:, st, :], ps[:, :],
                                            1.0 / 16)
                nc.vector.memset(v_sb[:, st, 64:320:80], 1.0)

            ckT_sb = actp.tile([128, 2, S], BF16, tag="ckT")
            proj_T(ckT_sb, ck_sb, xT_t)

            cv_nat = actp.tile([128, NJT, BAND], BF16, tag="cvN")
            for st in range(NJT):
                ps = psA.tile([128, BAND], F32, tag="acc")
                for kt in range(NKT):
                    nc.tensor.matmul(ps[:, :],
                                     xT_t[kt][:, st * 128:(st + 1) * 128],
                                     cv_sb[:, kt, :],
                                     start=(kt == 0), stop=(kt == NKT - 1))
                nc.vector.tensor_copy(cv_nat[:, st, :], ps[:, :])

            # meta weights: load now (slots of wq/wk/wv just freed)
            mqf = load_w("mqf", mqT, BAND, "wq", F8)
            mkf = load_w("mkf", mkT, BAND, "wk", F8)
            mvf = load_w("mvf", mvT, BAND, "wv", F8)

            # ---------- chunked tiles ----------
            ctxm_sb = actp.tile([128, 2, S], F8, tag="ctxm")  # 1.6*main ctx
            cA_sb = actp.tile([128, 2, S], BF16, tag="cA")
            ctxC = [actp.tile([128, NKT, IC], BF16, name=f"ctxC{i}",
                              tag=f"ctxC{i}") for i in range(NIC)]
            # fp8 ctx (x8) for the meta projections; chunks 2/3 reuse the
            # xf8 slots (dead after the last q/gate projections)
            ctxF = [actp.tile([128, NKT, IC], F8, name=f"ctxF{i}",
                              tag=("ctxF0", "ctxF1", "xf80", "xf81")[i])
                    for i in range(NIC)]
            bandC = [actp.tile([128, 2, IC], BF16, name=f"bandC{i}",
                               tag=f"bandC{i}") for i in range(NIC)]
            mqT_f8 = actp.tile([128, 2, S], F8, tag="qT2")
            mkT_f8 = actp.tile([128, 2, S], F8, tag="kT2")
            mv_nat = actp.tile([128, NJT, BAND], F8, tag="vA2")

            arB, rsO, agO, arO3 = [], [], [], []
            for icc in range(NIC):
                nh = 1 if icc < NIC - 1 else 2
                arB.append([dram.tile([D, IC // nh], BF16,
                                      name=f"arB{icc}_{hh}", tag=f"arB{icc}{hh}")
                            for hh in range(nh)])
                rsO.append([dram.tile([BAND, IC // nh], BF16,
                                      name=f"rsO{icc}_{hh}", tag=f"rsO{icc}{hh}")
                            for hh in range(nh)])
                if icc == NIC - 1:
                    arO3.extend([dram.tile([D, IC // nh], BF16,
                                           name=f"arO3_{hh}", tag=f"arO3{hh}")
                                 for hh in range(nh)])
                agO.append([dram.tile([D, IC // nh], BF16,
                                      name=f"agO{icc}_{hh}", tag=f"agO{icc}{hh}")
                            for hh in range(nh)])

            def div_batch(specs, i0):
                """softmax divisions: DVE fast-reciprocal -> PE ones-matmul
                broadcast -> DVE multiply with the spec's scale folded in.
                spec: ("head", h, acc) or ("wide", dst_sb, a1, a2, rs, sc)"""
                rcps = []
                for sp in specs:
                    rcp = statp.tile([1, IC], F32, tag="lnr", bufs=2)
                    if sp[0] == "head":
                        # reciprocal_approx_fast (custom DVE ucode) ignores
                        # the base partition — relocate rs to partition 0
                        rs0 = statp.tile([1, IC], F32, tag="rs0", bufs=2)
                        nc.vector.tensor_copy(rs0[:, :], sp[2][64:65, :])
                        src = rs0[:, :]
                    else:
                        src = sp[4][:, :]
                    nc.vector.reciprocal_approx_fast(rcp[:, :], src)
                    rcpb = statp.tile([1, IC], BF16, tag="rcp", bufs=3)
                    nc.vector.tensor_copy(rcpb[:, :], rcp[:, :])
                    rcps.append(rcpb)
                pbs = []
                for sp, rcpb in zip(specs, rcps):
                    n = 64 if sp[0] == "head" else 128
                    if sp[0] == "head":
                        pb_ps = psE.tile([128, IC], F32, tag="eps")
                    else:
                        pb_ps = psR.tile([128, IC], F32, tag="rs")
                    nc.tensor.matmul(pb_ps[:, :], onesrow[0:1, :], rcpb[:, :])
                    pb = work.tile([n, IC], BF16,
                                   tag="pbm" if n == 64 else "pb2", bufs=3)
                    nc.vector.tensor_copy(pb[:, :], pb_ps[0:n, :])
                    pbs.append(pb)
                for sp, pb in zip(specs, pbs):
                    if sp[0] == "head":
                        h, acc = sp[1], sp[2]
                        rh, oh = (h % 2) * 64, h // 2
                        nc.vector.scalar_tensor_tensor(
                            ctxm_sb[rh:rh + 64, oh, i0:i0 + IC],
                            acc[0:64, :], 1.6, pb[:, :], MUL, MUL)
                    else:
                        dst_sb, a1, a2, sc = sp[1], sp[2], sp[3], sp[5]
                        nc.vector.scalar_tensor_tensor(
                            dst_sb[:, 0, i0:i0 + IC], a1[:, :], sc,
                            pb[:, :], MUL, MUL)
                        nc.vector.scalar_tensor_tensor(
                            dst_sb[:, 1, i0:i0 + IC], a2[:, :], sc,
                            pb[:, :], MUL, MUL)

            def main_pair_step(p, t, i0, accs):
                """jt pair (2t, 2t+1) for main heads (2p, 2p+1): fp8 scores
                per jt, then one DoubleRow AV per head over the pair"""
                oh = p
                r0 = (i0 // IC % 2) * IC
                esbs = [work.tile([128, 2, IC], F8, tag="esb2", bufs=4,
                                  name=f"esb{p}{t}{i0}{hh}")
                        for hh in range(2)]
                for dj in range(2):
                    jt = 2 * t + dj
                    for hh in range(2):
                        rh = hh * 64
                        eps = psE.tile([128, IC], F32, tag="eps")
                        nc.tensor.matmul(
                            eps[:, :],
                            kf8_sb[rh:rh + 64, oh, jt * 128:(jt + 1) * 128],
                            qs_sb[rh:rh + 64, oh, r0:r0 + IC])
                        nc.scalar.activation(esbs[hh][:, dj, :], eps[:, :],
                                             AF.Exp, scale=0.125)
                for hh in range(2):
                    h = 2 * p + hh
                    nc.tensor.matmul(
                        accs[hh][:, :],
                        v_sb[:, 2 * t:2 * t + 2, 80 * h:80 * h + 65],
                        esbs[hh][:, 0:2, :],
                        start=(t == 0), stop=(t == NPR - 1), perf_mode=DR)

            def wide_attn_step(kTt, qTt, vnat, jt, i0, a1, a2, rs):
                """one j-tile of the bf16 hd-256 causal attention"""
                eps = psE.tile([128, IC], F32, tag="eps")
                for dkt in range(2):
                    nc.tensor.matmul(
                        eps[:, :],
                        kTt[:, dkt, jt * 128:(jt + 1) * 128],
                        qTt[:, dkt, i0:i0 + IC],
                        start=(dkt == 0), stop=(dkt == 1))
                esb = work.tile([128, IC], BF16, tag="esb", bufs=4)
                nc.scalar.activation(esb[:, :], eps[:, :], AF.Exp)
                st_, sp_ = (jt == 0), (jt == NJT - 1)
                nc.tensor.matmul(a1[:, :], vnat[:, jt, 0:128], esb[:, :],
                                 start=st_, stop=sp_)
                nc.tensor.matmul(a2[:, :], vnat[:, jt, 128:256], esb[:, :],
                                 start=st_, stop=sp_)
                nc.tensor.matmul(rs[:, :], ones_sb[:, 0:1], esb[:, :],
                                 start=st_, stop=sp_)

            def meta_attn_step(t, i0, a1, a2, rs):
                """jt pair (2t, 2t+1) of the fp8 DoubleRow meta attention"""
                esbm = work.tile([128, 2, IC], F8, tag="esbm", bufs=3)
                for dj in range(2):
                    jt = 2 * t + dj
                    eps = psE.tile([128, IC], F32, tag="eps")
                    nc.tensor.matmul(
                        eps[:, :],
                        mkT_f8[:, 0:2, jt * 128:(jt + 1) * 128],
                        mqT_f8[:, 0:2, i0:i0 + IC], perf_mode=DR)
                    nc.scalar.activation(esbm[:, dj, :], eps[:, :],
                                         AF.Exp, scale=1.0 / 256)
                st_, sp_ = (t == 0), (t == NPR - 1)
                nc.tensor.matmul(a1[:, :], mv_nat[:, 2 * t:2 * t + 2, 0:128],
                                 esbm[:, 0:2, :], start=st_, stop=sp_,
                                 perf_mode=DR)
                nc.tensor.matmul(a2[:, :], mv_nat[:, 2 * t:2 * t + 2, 128:256],
                                 esbm[:, 0:2, :], start=st_, stop=sp_,
                                 perf_mode=DR)
                nc.tensor.matmul(rs[:, :], ones_pair[:, 0:2, 0:1],
                                 esbm[:, 0:2, :], start=st_, stop=sp_,
                                 perf_mode=DR)

            def metaproj_steps(icc):
                """closures emitting chunk icc's meta projections (fp8 DR)"""
                i0 = icc * IC
                steps = []

                def caststep(icc=icc):
                    for kt in range(NKT):
                        nc.vector.tensor_scalar_mul(
                            ctxF[icc][:, kt, :], ctxC[icc][:, kt, :], 8.0)

                def projstep(w_f8, dst, ot, i0=i0, icc=icc):
                    ps = psA.tile([128, IC], F32, tag="acc")
                    for kp in range(4):
                        nc.tensor.matmul(
                            ps[:, :],
                            w_f8[:, 2 * kp:2 * kp + 2,
                                 ot * 128:(ot + 1) * 128],
                            ctxF[icc][:, 2 * kp:2 * kp + 2, :],
                            start=(kp == 0), stop=(kp == 3), perf_mode=DR)
                    nc.vector.tensor_scalar_mul(dst[:, ot, i0:i0 + IC],
                                                ps[:, :], 2.0 ** -5)

                def vstep(st4, icc=icc):
                    st = icc * 4 + st4
                    ps = psA.tile([128, BAND], F32, tag="acc")
                    for kp in range(4):
                        nc.tensor.matmul(
                            ps[:, :],
                            ctxF[icc][:, 2 * kp:2 * kp + 2,
                                      st4 * 128:(st4 + 1) * 128],
                            mvf[:, 2 * kp:2 * kp + 2, 0:BAND],
                            start=(kp == 0), stop=(kp == 3), perf_mode=DR)
                    nc.vector.tensor_scalar_mul(mv_nat[:, st, :], ps[:, :],
                                                2.0 ** -5)

                steps.append(caststep)
                for ot in range(2):
                    steps.append(lambda ot=ot: projstep(mqf, mqT_f8, ot))
                for ot in range(2):
                    steps.append(lambda ot=ot: projstep(mkf, mkT_f8, ot))
                for st4 in range(4):
                    steps.append(lambda st4=st4: vstep(st4))
                return steps

            # ---------- per-chunk pipeline ----------
            pending_rdback = []
            for icc in range(NIC):
                i0 = icc * IC
                while pending_rdback:
                    pending_rdback.pop(0)()
                # phase 1: main heads (0,1) interleaved with causal attention
                accA = [psA.tile([65, IC], F32, tag="acc", name=f"accA{icc}{i}")
                        for i in range(2)]
                ca1 = psA.tile([128, IC], F32, tag="acc")
                ca2 = psA.tile([128, IC], F32, tag="acc")
                crs = psR.tile([1, IC], F32, tag="rs")
                for t in range(NPR):
                    main_pair_step(0, t, i0, accA)
                    wide_attn_step(ckT_sb, cqT_sb, cv_nat, 2 * t, i0,
                                   ca1, ca2, crs)
                    wide_attn_step(ckT_sb, cqT_sb, cv_nat, 2 * t + 1, i0,
                                   ca1, ca2, crs)
                div_batch([("head", 0, accA[0]), ("head", 1, accA[1]),
                           ("wide", cA_sb, ca1, ca2, crs, 1.0)], i0)
                pe_keepwarm()

                # phase 2: main heads (2,3) interleaved with filler PE work:
                # next chunk's q/cq projections + older chunk's meta projs
                accB = [psA.tile([65, IC], F32, tag="acc", name=f"accB{icc}{i}")
                        for i in range(2)]
                fillers = []
                if icc + 1 < NIC:
                    fillers += qproj_steps(icc + 1)
                if icc >= 2:
                    fillers += metaproj_steps(icc - 2)
                nfront = len(fillers)
                done = 0
                for t in range(NPR):
                    main_pair_step(1, t, i0, accB)
                    want = (t + 1) * nfront // NPR
                    while done < want:
                        fillers[done]()
                        done += 1
                div_batch([("head", 2, accB[0]), ("head", 3, accB[1])], i0)
                pe_keepwarm()

                # causal out-proj + DoubleRow main placement -> arB chunk
                for ot in range(8):
                    ps = psA.tile([128, IC], F32, tag="acc")
                    for ft in range(2):
                        nc.tensor.matmul(
                            ps[:, :],
                            cow_sb[:, ft, ot * 128:(ot + 1) * 128],
                            cA_sb[:, ft, i0:i0 + IC],
                            start=(ft == 0), stop=False)
                    nc.tensor.matmul(
                        ps[:, :],
                        pcf[:, 0:2, ot * 128:(ot + 1) * 128],
                        ctxm_sb[:, 0:2, i0:i0 + IC],
                        start=False, stop=True, perf_mode=DR)
                    ob = work.tile([128, IC], BF16, tag="obA", bufs=2)
                    nc.vector.tensor_copy(ob[:, :], ps[:, :])
                    nh = len(arB[icc])
                    hw_ = IC // nh
                    for hh in range(nh):
                        nc.sync.dma_start(
                            arB[icc][hh][ot * 128:(ot + 1) * 128, :],
                            ob[:, hh * hw_:(hh + 1) * hw_])
                    if done < len(fillers) and ot % 2 == 1:
                        fillers[done]()
                        done += 1

                # blend combine: RS (own band) + AG (full ctx); the last
                # chunk is split in half so its latency tail is shorter.
                # Read-back DMAs are DEFERRED one block so the sync queue
                # never parks on an unfinished collective (head-of-line).
                nh = len(arB[icc])
                hw_ = IC // nh
                for hh in range(nh):
                    c0 = hh * hw_
                    if icc < NIC - 1:
                        nc.gpsimd.collective_compute(
                            "ReduceScatter", mybir.AluOpType.add,
                            replica_groups=groups,
                            ins=[arB[icc][hh][:, :].opt()],
                            outs=[rsO[icc][hh][:, :].opt()])
                        nc.gpsimd.collective_compute(
                            "AllGather", mybir.AluOpType.bypass,
                            replica_groups=groups,
                            ins=[rsO[icc][hh][:, :].opt()],
                            outs=[agO[icc][hh][:, :].opt()])
                    else:
                        nc.gpsimd.collective_compute(
                            "AllReduce", mybir.AluOpType.add,
                            replica_groups=groups,
                            ins=[arB[icc][hh][:, :].opt()],
                            outs=[arO3[hh][:, :].opt()])

                    def rdback(icc=icc, hh=hh, c0=c0, hw_=hw_):
                        src = agO[icc][hh] if icc < NIC - 1 else arO3[hh]
                        for kt in range(NKT):
                            nc.sync.dma_start(
                                ctxC[icc][:, kt, c0:c0 + hw_],
                                src[kt * 128:(kt + 1) * 128, :])
                        if icc < NIC - 1:
                            for kt in range(2):
                                nc.sync.dma_start(
                                    bandC[icc][:, kt, c0:c0 + hw_],
                                    rsO[icc][hh][kt * 128:(kt + 1) * 128, :])
                    pending_rdback.append(rdback)

            # meta projections for the last two chunks (chunk 2 overlaps
            # the RS3/AG3 tail; chunk 3 is emitted inside the first meta
            # attention chunk below, after its early j-tile pairs)
            while pending_rdback:
                pending_rdback.pop(0)()
            for st in metaproj_steps(NIC - 2):
                st()

            # ---------- meta attention + final out-proj ----------
            def final_steps(icc):
                i0 = icc * IC
                steps = []

                def fstep(ot, icc=icc, i0=i0):
                    ps = psA.tile([128, IC], F32, tag="acc")
                    for ft in range(2):
                        nc.tensor.matmul(
                            ps[:, :],
                            mow_sb[:, ft, ot * 128:(ot + 1) * 128],
                            mA_sb[:, ft, i0:i0 + IC],
                            start=(ft == 0), stop=False)
                    for ft in range(2):
                        nc.tensor.matmul(
                            ps[:, :],
                            ow_sb[:, ft, ot * 128:(ot + 1) * 128],
                            bandC[icc][:, ft, :],
                            start=False, stop=(ft == 1))
                    ob = work.tile([128, IC], F32, tag="obF", bufs=2)
                    nc.vector.tensor_copy(ob[:, :], ps[:, :])
                    nc.sync.dma_start(
                        outP[ot * 128:(ot + 1) * 128, i0:i0 + IC], ob[:, :])
                for ot in range(8):
                    steps.append(lambda ot=ot: fstep(ot))
                return steps

            mA_sb = actp.tile([128, 2, S], BF16, tag="cqT")  # reuse slot
            for icc in range(NIC):
                i0 = icc * IC
                fsteps = final_steps(icc - 1) if icc > 0 else []
                a1 = psA.tile([128, IC], F32, tag="acc")
                a2 = psA.tile([128, IC], F32, tag="acc")
                rs = psR.tile([1, IC], F32, tag="rs")
                # pairs 0..5 only touch chunks 0-2 of mk/mv, so the first
                # i-chunk's early pairs run while chunk 3's AG completes;
                # chunk 3's meta projections emit before the last 2 pairs
                for t in range(6):
                    meta_attn_step(t, i0, a1, a2, rs)
                    if fsteps and t < len(fsteps):
                        fsteps[t]()
                if icc == 0:
                    psel_sb = load_w("psel_sb", pselT, BAND, "cq")
                    for st in metaproj_steps(NIC - 1):
                        st()
                    for rt in range(2):
                        ps = psA.tile([128, IC], F32, tag="acc")
                        for kt in range(NKT):
                            nc.tensor.matmul(
                                ps[:, :],
                                psel_sb[:, kt, rt * 128:(rt + 1) * 128],
                                ctxC[NIC - 1][:, kt, :],
                                start=(kt == 0), stop=(kt == NKT - 1))
                        nc.vector.tensor_copy(bandC[NIC - 1][:, rt, :],
                                              ps[:, :])
                for t in range(6, NPR):
                    meta_attn_step(t, i0, a1, a2, rs)
                    if fsteps and t < len(fsteps):
                        fsteps[t]()
                div_batch([("wide", mA_sb, a1, a2, rs, 0.25)], i0)
                pe_keepwarm()

            for st in final_steps(NIC - 1):
                st()

            if DEBUG:
                for nm, t in [
                    ("d_mrow4", mrow4), ("d_kf8", kf8_sb), ("d_vsb", v_sb),
                    ("d_ctxm", ctxm_sb), ("d_cA", cA_sb),
                    ("d_ctxC0", ctxC[0]), ("d_ctxF0", ctxF[0]),
                    ("d_mq", mqT_f8), ("d_mk", mkT_f8), ("d_mv", mv_nat),
                    ("d_mA", mA_sb), ("d_bandC0", bandC[0]),
                    ("d_qs", qs_sb),
                ]:
                    ap = dbg[nm]
                    if len(t.shape) == 2:
                        nc.sync.dma_start(ap[:, :], t[:, :])
                    else:
                        nc.sync.dma_start(ap[:, :, :], t[:, :, :])

    nc.compile()
    return nc


_NC = None


def _get_nc():
    global _NC
    if _NC is None:
        _NC = build_program()
    return _NC


def kernel(hidden_states, consciousness_vector, wq, bq, wk, bk, wv, bv,
           gate_w, gate_b, aw_w, aw_b,
           causal_in_w, causal_in_b, causal_out_w, causal_out_b,
           meta_in_w, meta_in_b, meta_out_w, meta_out_b,
           out_w, out_b):
    f = np.float32
    hs = np.asarray(hidden_states, f)
    aw = np.asarray(consciousness_vector, f) @ np.asarray(aw_w, f).T \
        + np.asarray(aw_b, f)
    wfused = np.asarray(meta_out_w, f).T @ np.asarray(out_w, f).T  # [D, D]
    xTs = [np.ascontiguousarray(hs[b].T) for b in range(B)]

    def bfT(a):  # transpose + bf16
        return np.ascontiguousarray(np.asarray(a, f).T).astype(BF)

    def f8T(a, scale=16.0):  # transpose + scale + fp8
        return np.ascontiguousarray(np.asarray(a, f).T * scale).astype(F8NP)

    in_maps = []
    for c in range(NCORES):
        b, g = c // G, c % G
        sl = slice(g * BAND, (g + 1) * BAND)
        wv_aug = np.zeros((D, 320), f)
        for h in range(4):
            wv_aug[:, h * 80:h * 80 + 64] = \
                16.0 * np.asarray(wv, f)[g * BAND + h * 64:
                                         g * BAND + (h + 1) * 64].T
        gw_aug = np.zeros((D, 16), f)
        gw_aug[:, 0:4] = 16.0 * np.asarray(gate_w, f)[4 * g:4 * g + 4].T
        sel4 = np.zeros((4, 512), f)
        for h in range(4):
            sel4[h, h * 128:(h + 1) * 128] = 1.0
        sel4 = sel4.astype(BF)
        pc = np.zeros((BAND, D), f)
        pc[np.arange(BAND), g * BAND + np.arange(BAND)] = 0.0625
        psel = np.zeros((D, BAND), f)
        psel[g * BAND + np.arange(BAND), np.arange(BAND)] = 1.0
        in_maps.append({
            "xT": xTs[b].astype(BF),
            "xf8T": xTs[b].astype(F8NP),
            "wqT": f8T(np.asarray(wq, f)[sl]),
            "wkT": f8T(np.asarray(wk, f)[sl]),
            "wvT": wv_aug.astype(F8NP),
            "gwT": gw_aug.astype(F8NP),
            "selT": sel4,
            "awc": np.ascontiguousarray(aw[4 * g:4 * g + 4].reshape(1, 4)),
            "cqT": bfT(np.asarray(causal_in_w, f)[0:D][sl] / 16.0),
            "ckT": bfT(np.asarray(causal_in_w, f)[D:2 * D][sl]),
            "cvT": bfT(np.asarray(causal_in_w, f)[2 * D:][sl]),
            "cowT": np.ascontiguousarray(
                CAUSAL_ACTIVE * np.asarray(causal_out_w, f).T[sl]).astype(BF),
            "pcT": pc.astype(F8NP),
            "pselT": psel.astype(BF),
            "mqT": f8T(np.asarray(meta_in_w, f)[0:D][sl]),
            "mkT": f8T(np.asarray(meta_in_w, f)[D:2 * D][sl]),
            "mvT": f8T(np.asarray(meta_in_w, f)[2 * D:][sl]),
            "mowT": np.ascontiguousarray(MW * wfused[sl]).astype(BF),
            "owT": np.ascontiguousarray(
                (1.0 - MW) * np.asarray(out_w, f).T[sl]).astype(BF),
        })

    nc = _get_nc()
    res = run_bass_kernel_spmd(nc, in_maps, core_ids=list(range(NCORES)))

    bias_row = (np.asarray(out_b, f)
                + MW * (np.asarray(meta_out_b, f) @ np.asarray(out_w, f).T))
    out = np.empty((B, S, D), f)
    for b in range(B):
        acc = np.zeros((D, S), f)
        for g in range(G):
            acc += res.results[b * G + g]["outP"]
        out[b] = acc.T + bias_row[None, :]
    return out
